# revision 1
# baseline (speedup 1.0000x reference)
"""Trainium2 Bass kernel for windowed channel-attention (nn_ChannelAttention2).

Reference computation (per batch element b):
    qkv = x @ w_qkv                    # [L, 3C], L = 36864, C = 192
    per 64-token window w:
        q, k, v = qkv[w]               # [64, C] each
        attn = softmax_d(scale * k^T v)    # [C, C] (softmax over last axis d)
        out[w] = q @ attn^T            # [64, C]
    y = out @ w_proj + b_proj

Sharding: data-parallel over batch B=8 -> one batch element per NeuronCore.
All weights replicated. No collectives.

Hardware notes driving the design (measured on this stack):
  - concurrent row-tiled matmuls (tile_position rows 64) wedge the device;
    col-tiled matmuls give wrong results -> every matmul operand and output
    lives at base partition 0, no tile_position anywhere.
  - DMA cannot touch PSUM; PSUM->SBUF moves go through ScalarE/VectorE.
  - k/v need to be window-major (64-token windows at partitions 0:64) for the
    attention contraction; they are produced token-tile-major (128 tokens), so
    an SBUF->SBUF DMA repacks them (DMA engines are otherwise idle).

Per-core dataflow (token tile = 128 tokens = 2 windows):
    xT  [C, tok]   via PE transpose
    k,v [tok, C]   via matmul (xT stationary), repacked to windows via DMA
    qT  [C, tok]   via matmul (w_q stationary; lo block widened to M=128)
    attnT[d, c] = v^T k per window; E = exp(scale*attnT) on ScalarE
    D[c] = ones^T E (replicated over 64 partitions) ; 1/D via fast reciprocal
    out_raw[n, c] = qT^T E ; divide fused into the PSUM->SBUF copy
    outT via PE transpose ; y = outT^T @ w_proj + b (bias fused in final copy)
"""

import os

import numpy as np

C = 192
L = 36864
N_CORES = 8
WIN = 64
TOK_TILE = 128  # 2 windows
CH = 4          # token tiles per chunk (DMA/batching granularity)
SCALE = float((C // 8) ** -0.5)

_CACHE = {}


def _build(length=L, n_cores=N_CORES):
    import concourse.bass as bass
    import concourse.mybir as mybir
    import concourse.tile as tile
    from concourse import bacc
    from concourse.masks import make_identity

    f32 = mybir.dt.float32
    AF = mybir.ActivationFunctionType

    n_tiles = length // TOK_TILE
    n_chunks = n_tiles // CH
    assert n_chunks * CH == n_tiles

    nc = bacc.Bacc("TRN2", target_bir_lowering=False, debug=False,
                   num_devices=n_cores)
    x_d = nc.declare_dram_parameter("x", [length, C], f32, isOutput=False)
    wqkv_d = nc.declare_dram_parameter("w_qkv", [C, 3 * C], f32, isOutput=False)
    wp_d = nc.declare_dram_parameter("w_proj", [C, C], f32, isOutput=False)
    bp_d = nc.declare_dram_parameter("b_proj", [C], f32, isOutput=False)
    y_d = nc.declare_dram_parameter("y", [length, C], f32, isOutput=True)

    with tile.TileContext(nc) as tc:
        with (
            tc.tile_pool(name="singles", bufs=1) as singles,
            tc.tile_pool(name="sb", bufs=2) as sb,
            tc.tile_pool(name="ps2", bufs=2, space="PSUM") as ps2,
            tc.tile_pool(name="ps1", bufs=1, space="PSUM") as ps1,
        ):
            # ---- constants / weights (loaded once) ----
            ident = singles.tile([128, 128], f32)
            make_identity(nc, ident)
            ones_sb = singles.tile([128, 64], f32)
            nc.vector.memset(ones_sb, 1.0)
            wqkv_hi = singles.tile([128, 3 * C], f32)
            nc.sync.dma_start(out=wqkv_hi, in_=wqkv_d[0:128, :])
            wqkv_lo = singles.tile([64, 3 * C], f32)
            nc.sync.dma_start(out=wqkv_lo, in_=wqkv_d[128:192, :])
            wp_hi = singles.tile([128, C], f32)
            nc.sync.dma_start(out=wp_hi, in_=wp_d[0:128, :])
            wp_lo = singles.tile([64, C], f32)
            nc.sync.dma_start(out=wp_lo, in_=wp_d[128:192, :])
            b_sb = singles.tile([128, C], f32)
            nc.gpsimd.dma_start(
                out=b_sb,
                in_=bass.AP(tensor=bp_d, offset=0, ap=[[0, 128], [1, C]]))

            for ci in range(n_chunks):
                row0 = ci * CH * TOK_TILE
                x_chunk = x_d[row0:row0 + CH * TOK_TILE, :].rearrange(
                    "(t p) c -> p t c", p=TOK_TILE)
                y_chunk = y_d[row0:row0 + CH * TOK_TILE, :].rearrange(
                    "(t p) c -> p t c", p=TOK_TILE)

                x_sb = sb.tile([128, CH, 256], f32, tag="x_sb")
                nc.vector.memset(x_sb[:, :, 192:256], 0.0)
                nc.sync.dma_start(out=x_sb[:, :, 0:192], in_=x_chunk)

                xT_sb = sb.tile([128, CH, 256], f32, tag="xT_sb")
                kv_sb = sb.tile([128, CH, 2, C], f32, tag="kv_sb")
                kv2_sb = sb.tile([64, CH, 2, 2, C], f32, tag="kv2_sb")
                qT_sb = sb.tile([128, CH, 256], f32, tag="qT_sb")
                Eh_sb = sb.tile([128, CH, 2, C], f32, tag="Eh_sb")
                El_sb = sb.tile([64, CH, 2, C], f32, tag="El_sb")
                rD_sb = sb.tile([64, CH, 2, C], f32, tag="rD_sb")
                out_sb = sb.tile([64, CH, 2, 256], f32, tag="out_sb")
                nc.vector.memset(out_sb[:, :, :, 192:256], 0.0)
                outT_sb = sb.tile([128, CH, 256], f32, tag="outT_sb")
                fin_sb = sb.tile([128, CH, C], f32, tag="fin_sb")

                def front_half(t):
                    # ---- T1: x tile -> xT (PE transpose) ----
                    xq_ps = ps2.tile([128, 512], f32, tag="small")
                    nc.tensor.transpose(xq_ps[:, 0:128], x_sb[:, t, 0:128],
                                        ident)
                    nc.tensor.transpose(xq_ps[:, 128:256], x_sb[:, t, 128:256],
                                        ident)
                    nc.scalar.copy(xT_sb[:, t, 0:256], xq_ps[:, 0:256])
                    xT_hi = xT_sb[:, t, 0:128]       # [128 (e hi), 128 tok]
                    xT_lo = xT_sb[0:64, t, 128:256]  # [64 (e lo), 128 tok]

                    # ---- M_kv: k, v token-major ----
                    kv_ps = ps1.tile([128, 512], f32, tag="kv")
                    nc.tensor.matmul(kv_ps[:, 0:192], xT_hi,
                                     wqkv_hi[:, 192:384], start=True, stop=False)
                    nc.tensor.matmul(kv_ps[:, 0:192], xT_lo,
                                     wqkv_lo[:, 192:384], start=False, stop=True)
                    nc.tensor.matmul(kv_ps[:, 256:448], xT_hi,
                                     wqkv_hi[:, 384:576], start=True, stop=False)
                    nc.tensor.matmul(kv_ps[:, 256:448], xT_lo,
                                     wqkv_lo[:, 384:576], start=False, stop=True)
                    nc.vector.tensor_copy(
                        kv_sb[:, t, :, :],
                        kv_ps[:, 0:512].rearrange("p (s c) -> p s c",
                                                  s=2)[:, :, 0:192])

                    # ---- M_q: qT channel-major ----
                    # lo block widened to M=128 (weight cols 128:256) so the
                    # whole output region is written; q-lo sits at rows 0:64.
                    qT_ps = ps2.tile([128, 512], f32, tag="small")
                    nc.tensor.matmul(qT_ps[:, 0:128], wqkv_hi[:, 0:128], xT_hi,
                                     start=True, stop=False)
                    nc.tensor.matmul(qT_ps[:, 0:128], wqkv_lo[:, 0:128], xT_lo,
                                     start=False, stop=True)
                    nc.tensor.matmul(qT_ps[:, 128:256], wqkv_hi[:, 128:256],
                                     xT_hi, start=True, stop=False)
                    nc.tensor.matmul(qT_ps[:, 128:256], wqkv_lo[:, 128:256],
                                     xT_lo, start=False, stop=True)
                    nc.scalar.copy(qT_sb[:, t, 0:256], qT_ps[:, 0:256])

                def back_half(t):
                    # ---- attnT per window: attnT[d, c] = sum_n v[n,d] k[n,c]
                    # all operands window-major at partitions 0:64 ----
                    ah_ps = ps1.tile([128, 512], f32, tag="ah")
                    al_ps = ps1.tile([64, 512], f32, tag="al")
                    for w in range(2):
                        k_sl = kv2_sb[:, t, w, 0, :]        # [64, 192]
                        v_hi = kv2_sb[:, t, w, 1, 0:128]    # [64, 128]
                        v_lo = kv2_sb[:, t, w, 1, 128:192]  # [64, 64]
                        nc.tensor.matmul(ah_ps[:, 256 * w:256 * w + 192],
                                         v_hi, k_sl, start=True, stop=True)
                        nc.tensor.matmul(al_ps[:, 256 * w:256 * w + 192],
                                         v_lo, k_sl, start=True, stop=True)

                    # ---- exp (scale folded in) ----
                    nc.scalar.activation(
                        Eh_sb[:, t, :, :],
                        ah_ps[:, 0:512].rearrange("p (s c) -> p s c",
                                                  s=2)[:, :, 0:192],
                        AF.Exp, scale=SCALE)
                    nc.scalar.activation(
                        El_sb[:, t, :, :],
                        al_ps[:, 0:512].rearrange("p (s c) -> p s c",
                                                  s=2)[:, :, 0:192],
                        AF.Exp, scale=SCALE)

                    # ---- denominators, replicated across 64 partitions ----
                    d_ps = ps1.tile([64, 512], f32, tag="d")
                    for w in range(2):
                        nc.tensor.matmul(d_ps[:, 256 * w:256 * w + 192],
                                         ones_sb[0:128, :], Eh_sb[:, t, w, :],
                                         start=True, stop=False)
                        nc.tensor.matmul(d_ps[:, 256 * w:256 * w + 192],
                                         ones_sb[0:64, :], El_sb[:, t, w, :],
                                         start=False, stop=True)
                    nc.vector.reciprocal_approx_fast(
                        out=rD_sb[:, t, :, :],
                        in_=d_ps[:, 0:512].rearrange("p (s c) -> p s c",
                                                     s=2)[:, :, 0:192])

                    # ---- M_out: out_raw[n, c] = sum_d q[n,d] E[d,c] ----
                    out_ps = ps1.tile([64, 512], f32, tag="out")
                    for w in range(2):
                        n0 = w * WIN
                        nc.tensor.matmul(
                            out_ps[:, 256 * w:256 * w + 192],
                            qT_sb[:, t, n0:n0 + WIN],
                            Eh_sb[:, t, w, :], start=True, stop=False)
                        nc.tensor.matmul(
                            out_ps[:, 256 * w:256 * w + 192],
                            qT_sb[0:64, t, 128 + n0:128 + n0 + WIN],
                            El_sb[:, t, w, :], start=False, stop=True)

                    # ---- divide (fused into PSUM->SBUF copy) ----
                    nc.vector.tensor_mul(
                        out_sb[:, t, :, 0:192],
                        out_ps[:, 0:512].rearrange("p (s c) -> p s c",
                                                   s=2)[:, :, 0:192],
                        rD_sb[:, t, :, :])

                    # ---- T4 + proj + bias; outT shares the fin PSUM bank ----
                    finT_ps = ps1.tile([128, 512], f32, tag="finT")
                    for w in range(2):
                        nc.tensor.transpose(
                            finT_ps[:, 256 + 64 * w:320 + 64 * w],
                            out_sb[:, t, w, 0:128], ident[0:64, 0:64])
                        nc.tensor.transpose(
                            finT_ps[:, 384 + 64 * w:448 + 64 * w],
                            out_sb[:, t, w, 128:256], ident[0:64, 0:64])
                    nc.scalar.copy(outT_sb[:, t, 0:256], finT_ps[:, 256:512])

                    nc.tensor.matmul(finT_ps[:, 0:192], outT_sb[:, t, 0:128],
                                     wp_hi, start=True, stop=False)
                    nc.tensor.matmul(finT_ps[:, 0:192],
                                     outT_sb[0:64, t, 128:256],
                                     wp_lo, start=False, stop=True)
                    nc.vector.tensor_add(fin_sb[:, t, :], finT_ps[:, 0:192],
                                         b_sb)

                for tp in range(CH // 2):
                    t0, t1 = 2 * tp, 2 * tp + 1
                    front_half(t0)
                    front_half(t1)
                    # window-major repack of k/v for the pair (SBUF->SBUF DMA)
                    nc.sync.dma_start(out=kv2_sb[:, t0:t1 + 1, 0, :, :],
                                      in_=kv_sb[0:64, t0:t1 + 1, :, :])
                    nc.sync.dma_start(out=kv2_sb[:, t0:t1 + 1, 1, :, :],
                                      in_=kv_sb[64:128, t0:t1 + 1, :, :])
                    back_half(t0)
                    back_half(t1)

                nc.sync.dma_start(out=y_chunk, in_=fin_sb)

    nc.compile()
    return nc


def _get_nc(length=L, n_cores=N_CORES):
    key = (length, n_cores)
    if key not in _CACHE:
        _CACHE[key] = _build(length, n_cores)
    return _CACHE[key]


def kernel(x, w_qkv, w_proj, b_proj, H=None, W=None, **_unused):
    from concourse.bass_utils import run_bass_kernel_spmd

    x = np.asarray(x, dtype=np.float32)
    w_qkv = np.asarray(w_qkv, dtype=np.float32)
    w_proj = np.asarray(w_proj, dtype=np.float32)
    b_proj = np.asarray(b_proj, dtype=np.float32)
    B, length, c = x.shape
    assert B == N_CORES and c == C

    nc = _get_nc(length, N_CORES)
    in_maps = [
        {"x": np.ascontiguousarray(x[b]), "w_qkv": w_qkv, "w_proj": w_proj,
         "b_proj": b_proj}
        for b in range(B)
    ]
    res = run_bass_kernel_spmd(nc, in_maps, list(range(N_CORES)))
    return np.stack([res.results[b]["y"] for b in range(B)], axis=0)


if __name__ == "__main__":
    # mini smoke test: one chunk worth of tokens per core
    length = int(os.environ.get("K_LEN", CH * TOK_TILE))
    rng = np.random.default_rng(0)
    x = rng.standard_normal((N_CORES, length, C), dtype=np.float32)
    w_qkv = (rng.standard_normal((C, 3 * C)) * 0.02).astype(np.float32)
    w_proj = (rng.standard_normal((C, C)) * 0.02).astype(np.float32)
    b_proj = (rng.standard_normal((C,)) * 0.02).astype(np.float32)

    def ref(x):
        qkv = x @ w_qkv  # [B, L, 3C]
        B_, L_, _ = x.shape
        qkv = qkv.reshape(B_, L_ // 64, 64, 3, C)
        q, k, v = qkv[..., 0, :], qkv[..., 1, :] * SCALE, qkv[..., 2, :]
        attn = np.einsum('bwnc,bwnd->bwcd', k, v)
        attn = np.exp(attn - attn.max(-1, keepdims=True))
        attn = attn / attn.sum(-1, keepdims=True)
        out = np.einsum('bwcd,bwnd->bwnc', attn, q).reshape(B_, L_, C)
        return out @ w_proj + b_proj

    expected = ref(x)
    got = kernel(x, w_qkv, w_proj, b_proj)
    err = np.abs(got - expected).max()
    rel = np.abs(got - expected).max() / np.abs(expected).max()
    print(f"mini test: max abs err {err:.3e}  rel {rel:.3e}")



# revision 2
# speedup vs baseline: 1.0431x; 1.0431x over previous
"""Trainium2 Bass kernel v2 for windowed channel-attention (nn_ChannelAttention2).

Reference computation (per batch element b, one NeuronCore each):
    qkv = x @ w_qkv                    # [L, 3C], L = 36864, C = 192
    per 64-token window w:
        q, k, v = qkv[w]               # [64, C] each
        E = exp(scale * k^T v)         # [C, C]  (attnT[d, c], softmax over d)
        D[c] = sum_d E[d, c]
        out[w][n, c] = (sum_d q[n, d] E[d, c]) / D[c]
    y = out @ w_proj + b_proj

v2 changes vs baseline (all-fp32, 73 ms measured / 2.2 ms cost-model):
  - all matmul operands bf16 (fp32 is 4 cycles/row on the PE, bf16 is 1;
    rel-err budget 2e-2 >> bf16 error)
  - attention output computed directly transposed: outT[c, n] = E^T qT via
    lhsT=E (stationary), rhs=qT65 (moving) -> no separate out transposes
  - softmax denominator via ones-column appended to qT: outT[c, 64] = D[c]
    (c on partitions) -> reciprocal is a tiny op, normalization is a
    per-partition tensor_scalar multiply fused into the PSUM->SBUF copy
  - batched evacuations (one op per tile, strided APs) to amortize the
    per-op engine init bubble (ACT 172-222 cyc, DVE 58-120 cyc)
  - engine balance: exp + xT-evac on ACT; kv/qT evacs, recip, ts_mul, y on DVE

Matmul operands must live at partition base 0 (tile_position wedges this
stack), so k/v are repacked window-major via SBUF->SBUF DMA as in baseline.
"""

import os

import numpy as np

C = 192
L = 36864
N_CORES = 8
WIN = 64
TOK_TILE = 128  # 2 windows
CH = 8          # token tiles per chunk (DMA batching granularity)
SCALE = float((C // 8) ** -0.5)

_CACHE = {}


def _build(length=L, n_cores=N_CORES):
    import concourse.bass as bass
    import concourse.mybir as mybir
    import concourse.tile as tile
    from concourse import bacc
    from concourse.masks import make_identity

    f32 = mybir.dt.float32
    bf16 = mybir.dt.bfloat16
    AF = mybir.ActivationFunctionType

    n_tiles = length // TOK_TILE
    ch = min(CH, n_tiles)
    n_chunks = n_tiles // ch
    assert n_chunks * ch == n_tiles

    nc = bacc.Bacc("TRN2", target_bir_lowering=False, debug=False,
                   num_devices=n_cores)
    x_d = nc.declare_dram_parameter("x", [length, C], f32, isOutput=False)
    wqkv_d = nc.declare_dram_parameter("w_qkv", [C, 3 * C], f32, isOutput=False)
    wp_d = nc.declare_dram_parameter("w_proj", [C, C], f32, isOutput=False)
    bp_d = nc.declare_dram_parameter("b_proj", [C], f32, isOutput=False)
    y_d = nc.declare_dram_parameter("y", [length, C], f32, isOutput=True)

    with tile.TileContext(nc) as tc:
        with (
            tc.tile_pool(name="singles", bufs=1) as singles,
            tc.tile_pool(name="sb", bufs=2) as sb,
            tc.tile_pool(name="ps_a", bufs=1, space="PSUM") as ps_a,
            tc.tile_pool(name="ps_ot", bufs=2, space="PSUM") as ps_ot,
            tc.tile_pool(name="ps_kv", bufs=1, space="PSUM") as ps_kv,
            tc.tile_pool(name="ps_one", bufs=1, space="PSUM") as ps_one,
        ):
            # ---- constants / weights (loaded once, converted to bf16) ----
            ident = singles.tile([128, 128], f32)
            make_identity(nc, ident)

            wqkv_f32h = singles.tile([128, 3 * C], f32)
            nc.sync.dma_start(out=wqkv_f32h, in_=wqkv_d[0:128, :])
            wqkv_f32l = singles.tile([64, 3 * C], f32)
            nc.sync.dma_start(out=wqkv_f32l, in_=wqkv_d[128:192, :])
            wp_f32h = singles.tile([128, C], f32)
            nc.sync.dma_start(out=wp_f32h, in_=wp_d[0:128, :])
            wp_f32l = singles.tile([64, C], f32)
            nc.sync.dma_start(out=wp_f32l, in_=wp_d[128:192, :])

            wqkv_hi = singles.tile([128, 3 * C], bf16)
            nc.vector.tensor_copy(wqkv_hi, wqkv_f32h)
            wqkv_lo = singles.tile([64, 3 * C], bf16)
            nc.vector.tensor_copy(wqkv_lo, wqkv_f32l)
            wp_hi = singles.tile([128, C], bf16)
            nc.vector.tensor_copy(wp_hi, wp_f32h)
            wp_lo = singles.tile([64, C], bf16)
            nc.vector.tensor_copy(wp_lo, wp_f32l)

            b_sb = singles.tile([128, C], f32)
            nc.gpsimd.dma_start(
                out=b_sb,
                in_=bass.AP(tensor=bp_d, offset=0, ap=[[0, 128], [1, C]]))

            for ci in range(n_chunks):
                row0 = ci * ch * TOK_TILE
                x_chunk = x_d[row0:row0 + ch * TOK_TILE, :].rearrange(
                    "(t p) c -> p t c", p=TOK_TILE)
                y_chunk = y_d[row0:row0 + ch * TOK_TILE, :].rearrange(
                    "(t p) c -> p t c", p=TOK_TILE)

                x_sb = sb.tile([128, ch, 256], f32, tag="x_sb")
                nc.gpsimd.memset(x_sb[:, :, 192:256], 0.0)
                nc.sync.dma_start(out=x_sb[:, :, 0:192], in_=x_chunk)

                xT_sb = sb.tile([128, ch, 256], bf16, tag="xT_sb")
                kv_sb = sb.tile([128, ch, 2 * C], bf16, tag="kv_sb")
                kv2_sb = sb.tile([64, ch, 2, 448], bf16, tag="kv2_sb")
                nc.gpsimd.memset(kv2_sb[:, :, :, 384:448], 0.0)
                qT_sb = sb.tile([128, ch, 2, 66], bf16, tag="qT_sb")
                qTlo_sb = sb.tile([64, ch, 2, 66], bf16, tag="qTlo_sb")
                outT_sb = sb.tile([128, ch, 2, WIN], bf16, tag="outT_sb")
                outTlo_sb = sb.tile([64, ch, 2, WIN], bf16, tag="outTlo_sb")
                y_sb = sb.tile([128, ch, C], f32, tag="y_sb")

                # ones column for the denominator row of qT65
                nc.gpsimd.memset(qT_sb[:, :, :, 64:65], 1.0)
                nc.gpsimd.memset(qTlo_sb[:, :, :, 64:65], 1.0)

                def front_half(t0):
                    # pair-granular front half: tiles t0, t0+1
                    tsl = slice(t0, t0 + 2)
                    # ---- x tiles -> xT (PE transpose, fp32), evac converts ----
                    xq_ps = ps_a.tile([128, 2, 256], f32, tag="xqT")
                    for i in range(2):
                        nc.tensor.transpose(xq_ps[:, i, 0:128],
                                            x_sb[:, t0 + i, 0:128], ident)
                        nc.tensor.transpose(xq_ps[:, i, 128:256],
                                            x_sb[:, t0 + i, 128:256], ident)
                    nc.scalar.copy(xT_sb[:, tsl, :], xq_ps)

                    # ---- k, v token-major: out[tok, 2C] ----
                    kv_ps = ps_kv.tile([128, 2, 512], f32, tag="kv")
                    for i in range(2):
                        xT_hi = xT_sb[:, t0 + i, 0:128]
                        xT_lo = xT_sb[0:64, t0 + i, 128:256]
                        nc.tensor.matmul(kv_ps[:, i, 0:384], xT_hi,
                                         wqkv_hi[:, 192:576], start=True,
                                         stop=False)
                        nc.tensor.matmul(kv_ps[:, i, 0:384], xT_lo,
                                         wqkv_lo[:, 192:576], start=False,
                                         stop=True)
                    nc.vector.tensor_copy(kv_sb[:, tsl, :],
                                          kv_ps[:, :, 0:384])

                    # ---- qT channel-major: [e, tok]; e-lo at rows 0:64 ----
                    qT_ps = ps_a.tile([128, 2, 256], f32, tag="xqT")
                    for i in range(2):
                        xT_hi = xT_sb[:, t0 + i, 0:128]
                        xT_lo = xT_sb[0:64, t0 + i, 128:256]
                        nc.tensor.matmul(qT_ps[:, i, 0:128],
                                         wqkv_hi[:, 0:128], xT_hi,
                                         start=True, stop=False)
                        nc.tensor.matmul(qT_ps[:, i, 0:128],
                                         wqkv_lo[:, 0:128], xT_lo,
                                         start=False, stop=True)
                        nc.tensor.matmul(qT_ps[0:64, i, 128:256],
                                         wqkv_hi[:, 128:192], xT_hi,
                                         start=True, stop=False)
                        nc.tensor.matmul(qT_ps[0:64, i, 128:256],
                                         wqkv_lo[:, 128:192], xT_lo,
                                         start=False, stop=True)
                    # evac window-split: [e, t, 2, 64]
                    nc.vector.tensor_copy(
                        qT_sb[:, tsl, :, 0:64],
                        qT_ps[:, :, 0:128].rearrange(
                            "p t (w n) -> p t w n", w=2))
                    nc.vector.tensor_copy(
                        qTlo_sb[:, tsl, :, 0:64],
                        qT_ps[0:64, :, 128:256].rearrange(
                            "p t (w n) -> p t w n", w=2))

                def back_half(t):
                    # ---- attnT per window: E_ps[d, c] = sum_n v[n,d] k[n,c]
                    # hi/lo in separate banks for finer cross-tile overlap;
                    # lo rows 64:128 are zero-padded v columns ----
                    E_ps = ps_one.tile([128, 2, 512], f32, tag="E")
                    for w in range(2):
                        k_sl = kv2_sb[:, t, w, 0:192]
                        v_hi = kv2_sb[:, t, w, 192:320]
                        v_lo = kv2_sb[:, t, w, 320:448]
                        nc.tensor.matmul(E_ps[:, w, 0:192], v_hi, k_sl,
                                         start=True, stop=True)
                        nc.tensor.matmul(E_ps[:, w, 256:448], v_lo, k_sl,
                                         start=True, stop=True)

                    # ---- exp (scale folded in); one op covers hi+lo ----
                    E_sb = sb.tile([128, 2, 2, 192], bf16, tag="E_sb")
                    nc.scalar.activation(
                        E_sb,
                        E_ps.rearrange("p w (s c) -> p w s c",
                                       s=2)[:, :, :, 0:192],
                        AF.Exp, scale=SCALE)

                    # ---- outT[c, n] (+D at col 64) = E^T @ qT65 ----
                    oT_ps = ps_ot.tile([128, 2, 256], f32, tag="oT")
                    for w in range(2):
                        E_hi = E_sb[:, w, 0, :]      # [128 d-hi, 192]
                        E_lo = E_sb[0:64, w, 1, :]   # [64 d-lo, 192]
                        q65h = qT_sb[:, t, w, 0:65]
                        q65l = qTlo_sb[:, t, w, 0:65]
                        nc.tensor.matmul(oT_ps[:, w, 0:65], E_hi[:, 0:128],
                                         q65h, start=True, stop=False)
                        nc.tensor.matmul(oT_ps[:, w, 0:65], E_lo[:, 0:128],
                                         q65l, start=False, stop=True)
                        nc.tensor.matmul(oT_ps[0:64, w, 128:193],
                                         E_hi[:, 128:192], q65h, start=True,
                                         stop=False)
                        nc.tensor.matmul(oT_ps[0:64, w, 128:193],
                                         E_lo[:, 128:192], q65l, start=False,
                                         stop=True)

                    # ---- 1/D; then normalize via per-partition scalar ----
                    rD_sb = sb.tile([128, 2, 2], f32, tag="rD_sb")
                    nc.vector.reciprocal_approx_fast(
                        out=rD_sb[:, :, 0:1], in_=oT_ps[:, :, 64:65])
                    nc.vector.reciprocal_approx_fast(
                        out=rD_sb[0:64, :, 1:2], in_=oT_ps[0:64, :, 192:193])
                    for w in range(2):
                        nc.vector.tensor_scalar_mul(
                            outT_sb[:, t, w, :], oT_ps[:, w, 0:64],
                            rD_sb[:, w, 0:1])
                        nc.scalar.mul(
                            outTlo_sb[:, t, w, :], oT_ps[0:64, w, 128:192],
                            rD_sb[0:64, w, 1:2])

                    # ---- proj + bias ----
                    y_ps = ps_one.tile([128, 256], f32, tag="y")
                    nc.tensor.matmul(y_ps[:, 0:192],
                                     outT_sb[:, t, :, :], wp_hi,
                                     start=True, stop=False)
                    nc.tensor.matmul(y_ps[:, 0:192],
                                     outTlo_sb[:, t, :, :], wp_lo,
                                     start=False, stop=True)
                    nc.vector.tensor_add(y_sb[:, t, :], y_ps[:, 0:192], b_sb)

                for tp in range(ch // 2):
                    t0, t1 = 2 * tp, 2 * tp + 1
                    front_half(t0)
                    # window-major repack of k/v (SBUF->SBUF DMA)
                    nc.sync.dma_start(out=kv2_sb[:, t0:t1 + 1, 0, 0:384],
                                      in_=kv_sb[0:64, t0:t1 + 1, :])
                    nc.sync.dma_start(out=kv2_sb[:, t0:t1 + 1, 1, 0:384],
                                      in_=kv_sb[64:128, t0:t1 + 1, :])
                    back_half(t0)
                    back_half(t1)

                nc.sync.dma_start(out=y_chunk, in_=y_sb)

    nc.compile()
    return nc


def _get_nc(length=L, n_cores=N_CORES):
    key = (length, n_cores)
    if key not in _CACHE:
        _CACHE[key] = _build(length, n_cores)
    return _CACHE[key]


def kernel(x, w_qkv, w_proj, b_proj, H=None, W=None, **_unused):
    from concourse.bass_utils import run_bass_kernel_spmd

    x = np.asarray(x, dtype=np.float32)
    w_qkv = np.asarray(w_qkv, dtype=np.float32)
    w_proj = np.asarray(w_proj, dtype=np.float32)
    b_proj = np.asarray(b_proj, dtype=np.float32)
    B, length, c = x.shape
    assert B == N_CORES and c == C

    nc = _get_nc(length, N_CORES)
    in_maps = [
        {"x": np.ascontiguousarray(x[b]), "w_qkv": w_qkv, "w_proj": w_proj,
         "b_proj": b_proj}
        for b in range(B)
    ]
    res = run_bass_kernel_spmd(nc, in_maps, list(range(N_CORES)))
    return np.stack([res.results[b]["y"] for b in range(B)], axis=0)


def _np_ref(x, w_qkv, w_proj, b_proj):
    qkv = x @ w_qkv
    B_, L_, _ = x.shape
    qkv = qkv.reshape(B_, L_ // 64, 64, 3, C)
    q, k, v = qkv[..., 0, :], qkv[..., 1, :] * SCALE, qkv[..., 2, :]
    attn = np.einsum('bwnc,bwnd->bwcd', k, v)
    attn = np.exp(attn)
    attn = attn / attn.sum(-1, keepdims=True)
    out = np.einsum('bwcd,bwnd->bwnc', attn, q).reshape(B_, L_, C)
    return out @ w_proj + b_proj


if __name__ == "__main__":
    length = int(os.environ.get("K_LEN", 2 * CH * TOK_TILE))
    rng = np.random.default_rng(0)
    x = rng.standard_normal((N_CORES, length, C), dtype=np.float32)
    w_qkv = (rng.standard_normal((C, 3 * C)) * 0.02).astype(np.float32)
    w_proj = (rng.standard_normal((C, C)) * 0.02).astype(np.float32)
    b_proj = (rng.standard_normal((C,)) * 0.02).astype(np.float32)

    expected = _np_ref(x, w_qkv, w_proj, b_proj)
    got = kernel(x, w_qkv, w_proj, b_proj)
    err = np.abs(got - expected).max()
    rel = np.linalg.norm(got - expected) / np.linalg.norm(expected)
    print(f"mini test: max abs err {err:.3e}  rel_fro {rel:.3e}")


# revision 3
# speedup vs baseline: 1.0537x; 1.0102x over previous
"""Trainium2 Bass kernel v2 for windowed channel-attention (nn_ChannelAttention2).

Reference computation (per batch element b, one NeuronCore each):
    qkv = x @ w_qkv                    # [L, 3C], L = 36864, C = 192
    per 64-token window w:
        q, k, v = qkv[w]               # [64, C] each
        E = exp(scale * k^T v)         # [C, C]  (attnT[d, c], softmax over d)
        D[c] = sum_d E[d, c]
        out[w][n, c] = (sum_d q[n, d] E[d, c]) / D[c]
    y = out @ w_proj + b_proj

v2 changes vs baseline (all-fp32, 73 ms measured / 2.2 ms cost-model):
  - all matmul operands bf16 (fp32 is 4 cycles/row on the PE, bf16 is 1;
    rel-err budget 2e-2 >> bf16 error)
  - attention output computed directly transposed: outT[c, n] = E^T qT via
    lhsT=E (stationary), rhs=qT65 (moving) -> no separate out transposes
  - softmax denominator via ones-column appended to qT: outT[c, 64] = D[c]
    (c on partitions) -> reciprocal is a tiny op, normalization is a
    per-partition tensor_scalar multiply fused into the PSUM->SBUF copy
  - batched evacuations (one op per tile, strided APs) to amortize the
    per-op engine init bubble (ACT 172-222 cyc, DVE 58-120 cyc)
  - engine balance: exp + xT-evac on ACT; kv/qT evacs, recip, ts_mul, y on DVE

Matmul operands must live at partition base 0 (tile_position wedges this
stack), so k/v are repacked window-major via SBUF->SBUF DMA as in baseline.
"""

import os

import numpy as np

C = 192
L = 36864
N_CORES = 8
WIN = 64
TOK_TILE = 128  # 2 windows
CH = 8          # token tiles per chunk (DMA batching granularity)
SCALE = float((C // 8) ** -0.5)

_CACHE = {}


def _build(length=L, n_cores=N_CORES):
    import concourse.bass as bass
    import concourse.mybir as mybir
    import concourse.tile as tile
    from concourse import bacc
    from concourse.masks import make_identity

    f32 = mybir.dt.float32
    bf16 = mybir.dt.bfloat16
    AF = mybir.ActivationFunctionType

    n_tiles = length // TOK_TILE
    ch = min(CH, n_tiles)
    n_chunks = n_tiles // ch
    assert n_chunks * ch == n_tiles

    nc = bacc.Bacc("TRN2", target_bir_lowering=False, debug=False,
                   num_devices=n_cores)
    x_d = nc.declare_dram_parameter("x", [length, C], f32, isOutput=False)
    wqkv_d = nc.declare_dram_parameter("w_qkv", [C, 3 * C], f32, isOutput=False)
    wp_d = nc.declare_dram_parameter("w_proj", [C, C], f32, isOutput=False)
    bp_d = nc.declare_dram_parameter("b_proj", [C], f32, isOutput=False)
    y_d = nc.declare_dram_parameter("y", [length, C], f32, isOutput=True)

    with tile.TileContext(nc) as tc:
        with (
            tc.tile_pool(name="singles", bufs=1) as singles,
            tc.tile_pool(name="sb", bufs=2) as sb,
            tc.tile_pool(name="sbs", bufs=4) as sbs,
            tc.tile_pool(name="ps_a", bufs=1, space="PSUM") as ps_a,
            tc.tile_pool(name="ps_ot", bufs=2, space="PSUM") as ps_ot,
            tc.tile_pool(name="ps_kv", bufs=1, space="PSUM") as ps_kv,
            tc.tile_pool(name="ps_one", bufs=1, space="PSUM") as ps_one,
        ):
            # ---- constants / weights (loaded once, converted to bf16) ----
            ident = singles.tile([128, 128], f32)
            make_identity(nc, ident)

            wqkv_f32h = singles.tile([128, 3 * C], f32)
            nc.sync.dma_start(out=wqkv_f32h, in_=wqkv_d[0:128, :])
            wqkv_f32l = singles.tile([64, 3 * C], f32)
            nc.sync.dma_start(out=wqkv_f32l, in_=wqkv_d[128:192, :])
            wp_f32h = singles.tile([128, C], f32)
            nc.sync.dma_start(out=wp_f32h, in_=wp_d[0:128, :])
            wp_f32l = singles.tile([64, C], f32)
            nc.sync.dma_start(out=wp_f32l, in_=wp_d[128:192, :])

            wqkv_hi = singles.tile([128, 3 * C], bf16)
            nc.vector.tensor_copy(wqkv_hi, wqkv_f32h)
            wqkv_lo = singles.tile([64, 3 * C], bf16)
            nc.vector.tensor_copy(wqkv_lo, wqkv_f32l)
            wp_hi = singles.tile([128, C], bf16)
            nc.vector.tensor_copy(wp_hi, wp_f32h)
            wp_lo = singles.tile([64, C], bf16)
            nc.vector.tensor_copy(wp_lo, wp_f32l)

            b_sb = singles.tile([128, C], f32)
            nc.gpsimd.dma_start(
                out=b_sb,
                in_=bass.AP(tensor=bp_d, offset=0, ap=[[0, 128], [1, C]]))

            n_pairs_per_chunk = ch // 2
            n_pairs = n_tiles // 2
            chunk_state = {}

            def open_chunk(ci):
                row0 = ci * ch * TOK_TILE
                x_chunk = x_d[row0:row0 + ch * TOK_TILE, :].rearrange(
                    "(t p) c -> p t c", p=TOK_TILE)
                st = {}
                st["x_sb"] = sb.tile([128, ch, 256], f32, tag="x_sb")
                nc.gpsimd.memset(st["x_sb"][:, :, 192:256], 0.0)
                nc.gpsimd.dma_start(out=st["x_sb"][:, :, 0:192], in_=x_chunk)
                st["xT_sb"] = sb.tile([128, ch, 256], bf16, tag="xT_sb")
                st["kv_sb"] = sb.tile([128, ch, 2 * C], bf16, tag="kv_sb")
                st["kv2_sb"] = sb.tile([64, ch, 2, 448], bf16, tag="kv2_sb")
                nc.gpsimd.memset(st["kv2_sb"][:, :, :, 384:448], 0.0)
                st["qT_sb"] = sb.tile([128, ch, 2, 66], bf16, tag="qT_sb")
                st["qTlo_sb"] = sb.tile([64, ch, 2, 66], bf16, tag="qTlo_sb")
                st["outT_sb"] = sb.tile([128, ch, 2, WIN], bf16, tag="outT_sb")
                st["outTlo_sb"] = sb.tile([64, ch, 2, WIN], bf16,
                                          tag="outTlo_sb")
                st["y_sb"] = sb.tile([128, ch, C], f32, tag="y_sb")
                nc.gpsimd.memset(st["qT_sb"][:, :, :, 64:65], 1.0)
                nc.gpsimd.memset(st["qTlo_sb"][:, :, :, 64:65], 1.0)
                return st

            def front_half(st, t0):
                x_sb = st["x_sb"]
                xT_sb = st["xT_sb"]
                kv_sb = st["kv_sb"]
                qT_sb = st["qT_sb"]
                qTlo_sb = st["qTlo_sb"]
                tsl = slice(t0, t0 + 2)
                # ---- x tiles -> xT (PE transpose, fp32), evac converts ----
                xq_ps = ps_a.tile([128, 2, 256], f32, tag="xqT")
                for i in range(2):
                    nc.tensor.transpose(xq_ps[:, i, 0:128],
                                        x_sb[:, t0 + i, 0:128], ident)
                    nc.tensor.transpose(xq_ps[:, i, 128:256],
                                        x_sb[:, t0 + i, 128:256], ident)
                nc.scalar.copy(xT_sb[:, tsl, :], xq_ps)

                # ---- k, v token-major: out[tok, 2C]; per-tile evac +
                # repack so the attn pipeline starts sooner ----
                kv2_sb = st["kv2_sb"]
                for i in range(2):
                    kv_ps = ps_kv.tile([128, 512], f32, tag="kv")
                    xT_hi = xT_sb[:, t0 + i, 0:128]
                    xT_lo = xT_sb[0:64, t0 + i, 128:256]
                    nc.tensor.matmul(kv_ps[:, 0:384], xT_hi,
                                     wqkv_hi[:, 192:576], start=True,
                                     stop=False)
                    nc.tensor.matmul(kv_ps[:, 0:384], xT_lo,
                                     wqkv_lo[:, 192:576], start=False,
                                     stop=True)
                    nc.scalar.copy(kv_sb[:, t0 + i, :], kv_ps[:, 0:384])
                    nc.sync.dma_start(
                        out=kv2_sb[:, t0 + i, 0, 0:384],
                        in_=kv_sb[0:64, t0 + i, :])
                    nc.sync.dma_start(
                        out=kv2_sb[:, t0 + i, 1, 0:384],
                        in_=kv_sb[64:128, t0 + i, :])

                # ---- qT channel-major: [e, tok]; e-lo at rows 0:64 ----
                qT_ps = ps_a.tile([128, 2, 256], f32, tag="xqT")
                for i in range(2):
                    xT_hi = xT_sb[:, t0 + i, 0:128]
                    xT_lo = xT_sb[0:64, t0 + i, 128:256]
                    nc.tensor.matmul(qT_ps[:, i, 0:128],
                                     wqkv_hi[:, 0:128], xT_hi,
                                     start=True, stop=False)
                    nc.tensor.matmul(qT_ps[:, i, 0:128],
                                     wqkv_lo[:, 0:128], xT_lo,
                                     start=False, stop=True)
                    nc.tensor.matmul(qT_ps[0:64, i, 128:256],
                                     wqkv_hi[:, 128:192], xT_hi,
                                     start=True, stop=False)
                    nc.tensor.matmul(qT_ps[0:64, i, 128:256],
                                     wqkv_lo[:, 128:192], xT_lo,
                                     start=False, stop=True)
                nc.vector.tensor_copy(
                    qT_sb[:, tsl, :, 0:64],
                    qT_ps[:, :, 0:128].rearrange(
                        "p t (w n) -> p t w n", w=2))
                nc.vector.tensor_copy(
                    qTlo_sb[:, tsl, :, 0:64],
                    qT_ps[0:64, :, 128:256].rearrange(
                        "p t (w n) -> p t w n", w=2))


            def back_half(st, t):
                kv2_sb = st["kv2_sb"]
                qT_sb = st["qT_sb"]
                qTlo_sb = st["qTlo_sb"]
                outT_sb = st["outT_sb"]
                outTlo_sb = st["outTlo_sb"]
                y_sb = st["y_sb"]
                # ---- attnT per window: E_ps[d, c] = sum_n v[n,d] k[n,c] ----
                E_ps = ps_one.tile([128, 2, 512], f32, tag="E")
                for w in range(2):
                    k_sl = kv2_sb[:, t, w, 0:192]
                    v_hi = kv2_sb[:, t, w, 192:320]
                    v_lo = kv2_sb[:, t, w, 320:448]
                    nc.tensor.matmul(E_ps[:, w, 0:192], v_hi, k_sl,
                                     start=True, stop=True)
                    nc.tensor.matmul(E_ps[:, w, 256:448], v_lo, k_sl,
                                     start=True, stop=True)

                # ---- exp (scale folded in); one op covers hi+lo ----
                E_sb = sbs.tile([128, 2, 2, 192], bf16, tag="E_sb")
                nc.scalar.activation(
                    E_sb,
                    E_ps.rearrange("p w (s c) -> p w s c",
                                   s=2)[:, :, :, 0:192],
                    AF.Exp, scale=SCALE)

                # ---- outT[c, n] (+D at col 64) = E^T @ qT65 ----
                oT_ps = ps_ot.tile([128, 2, 256], f32, tag="oT")
                for w in range(2):
                    E_hi = E_sb[:, w, 0, :]      # [128 d-hi, 192]
                    E_lo = E_sb[0:64, w, 1, :]   # [64 d-lo, 192]
                    q65h = qT_sb[:, t, w, 0:65]
                    q65l = qTlo_sb[:, t, w, 0:65]
                    nc.tensor.matmul(oT_ps[:, w, 0:65], E_hi[:, 0:128],
                                     q65h, start=True, stop=False)
                    nc.tensor.matmul(oT_ps[:, w, 0:65], E_lo[:, 0:128],
                                     q65l, start=False, stop=True)
                    nc.tensor.matmul(oT_ps[0:64, w, 128:193],
                                     E_hi[:, 128:192], q65h, start=True,
                                     stop=False)
                    nc.tensor.matmul(oT_ps[0:64, w, 128:193],
                                     E_lo[:, 128:192], q65l, start=False,
                                     stop=True)

                # ---- 1/D; then normalize via broadcast tensor_mul ----
                rD_sb = sbs.tile([128, 2, 2], f32, tag="rD_sb")
                nc.vector.reciprocal_approx_fast(
                    out=rD_sb[:, :, 0:1], in_=oT_ps[:, :, 64:65])
                nc.vector.reciprocal_approx_fast(
                    out=rD_sb[0:64, :, 1:2], in_=oT_ps[0:64, :, 192:193])
                rd_h = rD_sb[:, :, 0:1]
                rd_h = bass.AP(tensor=rd_h.tensor, offset=rd_h.offset,
                               ap=[rd_h.ap[0], rd_h.ap[1], [0, WIN]])
                rd_l = rD_sb[0:64, :, 1:2]
                rd_l = bass.AP(tensor=rd_l.tensor, offset=rd_l.offset,
                               ap=[rd_l.ap[0], rd_l.ap[1], [0, WIN]])
                nc.vector.tensor_mul(outT_sb[:, t, :, :],
                                     oT_ps[:, :, 0:64], rd_h)
                nc.vector.tensor_mul(outTlo_sb[:, t, :, :],
                                     oT_ps[0:64, :, 128:192], rd_l)

                # ---- proj + bias ----
                y_ps = ps_one.tile([128, 256], f32, tag="y")
                nc.tensor.matmul(y_ps[:, 0:192],
                                 outT_sb[:, t, :, :], wp_hi,
                                 start=True, stop=False)
                nc.tensor.matmul(y_ps[:, 0:192],
                                 outTlo_sb[:, t, :, :], wp_lo,
                                 start=False, stop=True)
                nc.vector.tensor_add(y_sb[:, t, :], y_ps[:, 0:192], b_sb)

            def close_chunk(ci):
                st = chunk_state.pop(ci)
                row0 = ci * ch * TOK_TILE
                y_chunk = y_d[row0:row0 + ch * TOK_TILE, :].rearrange(
                    "(t p) c -> p t c", p=TOK_TILE)
                nc.gpsimd.dma_start(out=y_chunk, in_=st["y_sb"])

            def get_chunk(ci):
                if ci not in chunk_state:
                    chunk_state[ci] = open_chunk(ci)
                return chunk_state[ci]

            # software pipeline: front half runs one pair ahead
            front_half(get_chunk(0), 0)
            for p in range(n_pairs):
                ci, tp = divmod(p, n_pairs_per_chunk)
                if p + 1 < n_pairs:
                    ci2, tp2 = divmod(p + 1, n_pairs_per_chunk)
                    front_half(get_chunk(ci2), 2 * tp2)
                st = get_chunk(ci)
                back_half(st, 2 * tp)
                back_half(st, 2 * tp + 1)
                if tp == n_pairs_per_chunk - 1:
                    close_chunk(ci)

    nc.compile()
    return nc


def _get_nc(length=L, n_cores=N_CORES):
    key = (length, n_cores)
    if key not in _CACHE:
        _CACHE[key] = _build(length, n_cores)
    return _CACHE[key]


def kernel(x, w_qkv, w_proj, b_proj, H=None, W=None, **_unused):
    from concourse.bass_utils import run_bass_kernel_spmd

    x = np.asarray(x, dtype=np.float32)
    w_qkv = np.asarray(w_qkv, dtype=np.float32)
    w_proj = np.asarray(w_proj, dtype=np.float32)
    b_proj = np.asarray(b_proj, dtype=np.float32)
    B, length, c = x.shape
    assert B == N_CORES and c == C

    nc = _get_nc(length, N_CORES)
    in_maps = [
        {"x": np.ascontiguousarray(x[b]), "w_qkv": w_qkv, "w_proj": w_proj,
         "b_proj": b_proj}
        for b in range(B)
    ]
    res = run_bass_kernel_spmd(nc, in_maps, list(range(N_CORES)))
    return np.stack([res.results[b]["y"] for b in range(B)], axis=0)


def _np_ref(x, w_qkv, w_proj, b_proj):
    qkv = x @ w_qkv
    B_, L_, _ = x.shape
    qkv = qkv.reshape(B_, L_ // 64, 64, 3, C)
    q, k, v = qkv[..., 0, :], qkv[..., 1, :] * SCALE, qkv[..., 2, :]
    attn = np.einsum('bwnc,bwnd->bwcd', k, v)
    attn = np.exp(attn)
    attn = attn / attn.sum(-1, keepdims=True)
    out = np.einsum('bwcd,bwnd->bwnc', attn, q).reshape(B_, L_, C)
    return out @ w_proj + b_proj


if __name__ == "__main__":
    length = int(os.environ.get("K_LEN", 2 * CH * TOK_TILE))
    rng = np.random.default_rng(0)
    x = rng.standard_normal((N_CORES, length, C), dtype=np.float32)
    w_qkv = (rng.standard_normal((C, 3 * C)) * 0.02).astype(np.float32)
    w_proj = (rng.standard_normal((C, C)) * 0.02).astype(np.float32)
    b_proj = (rng.standard_normal((C,)) * 0.02).astype(np.float32)

    expected = _np_ref(x, w_qkv, w_proj, b_proj)
    got = kernel(x, w_qkv, w_proj, b_proj)
    err = np.abs(got - expected).max()
    rel = np.linalg.norm(got - expected) / np.linalg.norm(expected)
    print(f"mini test: max abs err {err:.3e}  rel_fro {rel:.3e}")


# revision 7
# speedup vs baseline: 30.1687x; 28.6304x over previous
"""Trainium2 Bass kernel v2 for windowed channel-attention (nn_ChannelAttention2).

Reference computation (per batch element b, one NeuronCore each):
    qkv = x @ w_qkv                    # [L, 3C], L = 36864, C = 192
    per 64-token window w:
        q, k, v = qkv[w]               # [64, C] each
        E = exp(scale * k^T v)         # [C, C]  (attnT[d, c], softmax over d)
        D[c] = sum_d E[d, c]
        out[w][n, c] = (sum_d q[n, d] E[d, c]) / D[c]
    y = out @ w_proj + b_proj

v2 changes vs baseline (all-fp32; true device time ~4 ms -> ~1.2 ms):
  - all matmul operands bf16 (fp32 is 4 cycles/row on the PE, bf16 is 1;
    rel-err budget 2e-2 >> bf16 error; measured rel_fro ~1e-3)
  - attention output computed directly transposed: outT[c, n] = E^T qT65 via
    lhsT=E (stationary), rhs=qT65 (moving) -> no separate out transposes
  - softmax denominator via ones-column appended to qT: outT[c, 64] = D[c]
    lands with c on partitions -> tiny reciprocal, then normalization is one
    tensor_mul per c-block with 1/D broadcast along tokens (0-stride AP),
    fused into the PSUM->SBUF evacuation
  - batched evacuations (one op per tile pair, strided APs) to amortize the
    per-op engine init bubble (ACT 172-222 cyc, DVE 58-120 cyc)
  - engine balance: exp + xT/kv evacs on ACT; qT evacs, recip, muls, y on DVE;
    memsets + chunk HBM DMAs on GPSIMD (SWDGE) keeping the SP FIFO free for
    the latency-critical k/v repacks
  - software-pipelined emission (front half of pair p+1 issued mid-pair p)
    to avoid head-of-line blocking in the strict-FIFO engine queues; PSUM
    banks: xq/qT rotation 2, kv 1, E 2, outT 2 (double-buffered), y 1

Matmul operands must live at partition base 0 (tile_position wedges this
stack), so k/v are repacked window-major via SBUF->SBUF DMA as in baseline.
tensor_tensor(divide) crashes the walrus birverifier -> reciprocal + mul.
"""

import os

import numpy as np

C = 192
L = 36864
N_CORES = 8
WIN = 64
TOK_TILE = 128  # 2 windows
CH = 8          # token tiles per chunk (DMA batching granularity)
SCALE = float((C // 8) ** -0.5)

_CACHE = {}


def _build(length=L, n_cores=N_CORES):
    import concourse.bass as bass
    import concourse.mybir as mybir
    import concourse.tile as tile
    from concourse import bacc
    from concourse.masks import make_identity

    f32 = mybir.dt.float32
    bf16 = mybir.dt.bfloat16
    AF = mybir.ActivationFunctionType

    n_tiles = length // TOK_TILE
    ch = min(CH, n_tiles)
    n_chunks = n_tiles // ch
    assert n_chunks * ch == n_tiles

    nc = bacc.Bacc("TRN2", target_bir_lowering=False, debug=False,
                   num_devices=n_cores)
    x_d = nc.declare_dram_parameter("x", [length, C], f32, isOutput=False)
    wqkv_d = nc.declare_dram_parameter("w_qkv", [C, 3 * C], f32, isOutput=False)
    wp_d = nc.declare_dram_parameter("w_proj", [C, C], f32, isOutput=False)
    bp_d = nc.declare_dram_parameter("b_proj", [C], f32, isOutput=False)
    y_d = nc.declare_dram_parameter("y", [length, C], f32, isOutput=True)

    with tile.TileContext(nc) as tc:
        with (
            tc.tile_pool(name="singles", bufs=1) as singles,
            tc.tile_pool(name="sb", bufs=2) as sb,
            tc.tile_pool(name="sbs", bufs=4) as sbs,
            tc.tile_pool(name="ps_a", bufs=1, space="PSUM") as ps_a,
            tc.tile_pool(name="ps_ot", bufs=2, space="PSUM") as ps_ot,
            tc.tile_pool(name="ps_kv", bufs=1, space="PSUM") as ps_kv,
            tc.tile_pool(name="ps_one", bufs=1, space="PSUM") as ps_one,
        ):
            # ---- constants / weights (loaded once, converted to bf16) ----
            ident = singles.tile([128, 128], f32)
            make_identity(nc, ident)

            wqkv_f32h = singles.tile([128, 3 * C], f32)
            nc.sync.dma_start(out=wqkv_f32h, in_=wqkv_d[0:128, :])
            wqkv_f32l = singles.tile([64, 3 * C], f32)
            nc.sync.dma_start(out=wqkv_f32l, in_=wqkv_d[128:192, :])
            wp_f32h = singles.tile([128, C], f32)
            nc.sync.dma_start(out=wp_f32h, in_=wp_d[0:128, :])
            wp_f32l = singles.tile([64, C], f32)
            nc.sync.dma_start(out=wp_f32l, in_=wp_d[128:192, :])

            wqkv_hi = singles.tile([128, 3 * C], bf16)
            nc.vector.tensor_copy(wqkv_hi, wqkv_f32h)
            wqkv_lo = singles.tile([64, 3 * C], bf16)
            nc.vector.tensor_copy(wqkv_lo, wqkv_f32l)
            wp_hi = singles.tile([128, C], bf16)
            nc.vector.tensor_copy(wp_hi, wp_f32h)
            wp_lo = singles.tile([64, C], bf16)
            nc.vector.tensor_copy(wp_lo, wp_f32l)

            b_sb = singles.tile([128, C], f32)
            nc.gpsimd.dma_start(
                out=b_sb,
                in_=bass.AP(tensor=bp_d, offset=0, ap=[[0, 128], [1, C]]))

            n_pairs_per_chunk = ch // 2
            n_pairs = n_tiles // 2
            chunk_state = {}

            def open_chunk(ci):
                row0 = ci * ch * TOK_TILE
                x_chunk = x_d[row0:row0 + ch * TOK_TILE, :].rearrange(
                    "(t p) c -> p t c", p=TOK_TILE)
                st = {}
                st["x_sb"] = sb.tile([128, ch, 256], f32, tag="x_sb", name="x_sb")
                nc.gpsimd.memset(st["x_sb"][:, :, 192:256], 0.0)
                nc.gpsimd.dma_start(out=st["x_sb"][:, :, 0:192], in_=x_chunk)
                st["xT_sb"] = sb.tile([128, ch, 256], bf16, tag="xT_sb", name="xT_sb")
                st["kv_sb"] = sb.tile([128, ch, 2 * C], bf16, tag="kv_sb", name="kv_sb")
                st["kv2_sb"] = sb.tile([64, ch, 2, 448], bf16, tag="kv2_sb", name="kv2_sb")
                nc.gpsimd.memset(st["kv2_sb"][:, :, :, 384:448], 0.0)
                st["qT_sb"] = sb.tile([128, ch, 2, 66], bf16, tag="qT_sb", name="qT_sb")
                st["qTlo_sb"] = sb.tile([64, ch, 2, 66], bf16, tag="qTlo_sb", name="qTlo_sb")
                st["outT_sb"] = sb.tile([128, ch, 2, WIN], bf16, tag="outT_sb", name="outT_sb")
                st["outTlo_sb"] = sb.tile([64, ch, 2, WIN], bf16,
                                          tag="outTlo_sb", name="outTlo_sb")
                st["y_sb"] = sb.tile([128, ch, C], f32, tag="y_sb", name="y_sb")
                nc.gpsimd.memset(st["qT_sb"][:, :, :, 64:65], 1.0)
                nc.gpsimd.memset(st["qTlo_sb"][:, :, :, 64:65], 1.0)
                return st

            def front_half(st, t0):
                x_sb = st["x_sb"]
                xT_sb = st["xT_sb"]
                kv_sb = st["kv_sb"]
                qT_sb = st["qT_sb"]
                qTlo_sb = st["qTlo_sb"]
                tsl = slice(t0, t0 + 2)
                # ---- x tiles -> xT (PE transpose, fp32), evac converts ----
                xq_ps = ps_a.tile([128, 2, 256], f32, tag="xqT")
                for i in range(2):
                    nc.tensor.transpose(xq_ps[:, i, 0:128],
                                        x_sb[:, t0 + i, 0:128], ident)
                    nc.tensor.transpose(xq_ps[:, i, 128:256],
                                        x_sb[:, t0 + i, 128:256], ident)
                nc.scalar.copy(xT_sb[:, tsl, :], xq_ps)

                # ---- k, v token-major: out[tok, 2C]; per-tile evac +
                # repack so the attn pipeline starts sooner ----
                kv2_sb = st["kv2_sb"]
                for i in range(2):
                    kv_ps = ps_kv.tile([128, 512], f32, tag="kv")
                    xT_hi = xT_sb[:, t0 + i, 0:128]
                    xT_lo = xT_sb[0:64, t0 + i, 128:256]
                    nc.tensor.matmul(kv_ps[:, 0:384], xT_hi,
                                     wqkv_hi[:, 192:576], start=True,
                                     stop=False)
                    nc.tensor.matmul(kv_ps[:, 0:384], xT_lo,
                                     wqkv_lo[:, 192:576], start=False,
                                     stop=True)
                    nc.scalar.copy(kv_sb[:, t0 + i, :], kv_ps[:, 0:384])
                    nc.sync.dma_start(
                        out=kv2_sb[:, t0 + i, 0, 0:384],
                        in_=kv_sb[0:64, t0 + i, :])
                    nc.sync.dma_start(
                        out=kv2_sb[:, t0 + i, 1, 0:384],
                        in_=kv_sb[64:128, t0 + i, :])

                # ---- qT channel-major: [e, tok]; e-lo at rows 0:64 ----
                qT_ps = ps_a.tile([128, 2, 256], f32, tag="xqT")
                for i in range(2):
                    xT_hi = xT_sb[:, t0 + i, 0:128]
                    xT_lo = xT_sb[0:64, t0 + i, 128:256]
                    nc.tensor.matmul(qT_ps[:, i, 0:128],
                                     wqkv_hi[:, 0:128], xT_hi,
                                     start=True, stop=False)
                    nc.tensor.matmul(qT_ps[:, i, 0:128],
                                     wqkv_lo[:, 0:128], xT_lo,
                                     start=False, stop=True)
                    nc.tensor.matmul(qT_ps[0:64, i, 128:256],
                                     wqkv_hi[:, 128:192], xT_hi,
                                     start=True, stop=False)
                    nc.tensor.matmul(qT_ps[0:64, i, 128:256],
                                     wqkv_lo[:, 128:192], xT_lo,
                                     start=False, stop=True)
                nc.vector.tensor_copy(
                    qT_sb[:, tsl, :, 0:64],
                    qT_ps[:, :, 0:128].rearrange(
                        "p t (w n) -> p t w n", w=2))
                nc.vector.tensor_copy(
                    qTlo_sb[:, tsl, :, 0:64],
                    qT_ps[0:64, :, 128:256].rearrange(
                        "p t (w n) -> p t w n", w=2))


            def back_half(st, t):
                kv2_sb = st["kv2_sb"]
                qT_sb = st["qT_sb"]
                qTlo_sb = st["qTlo_sb"]
                outT_sb = st["outT_sb"]
                outTlo_sb = st["outTlo_sb"]
                y_sb = st["y_sb"]
                # ---- attnT per window: E_ps[d, c] = sum_n v[n,d] k[n,c] ----
                E_ps = ps_one.tile([128, 2, 512], f32, tag="E")
                for w in range(2):
                    k_sl = kv2_sb[:, t, w, 0:192]
                    v_hi = kv2_sb[:, t, w, 192:320]
                    v_lo = kv2_sb[:, t, w, 320:448]
                    nc.tensor.matmul(E_ps[:, w, 0:192], v_hi, k_sl,
                                     start=True, stop=True)
                    nc.tensor.matmul(E_ps[:, w, 256:448], v_lo, k_sl,
                                     start=True, stop=True)

                # ---- exp (scale folded in); one op covers hi+lo ----
                E_sb = sbs.tile([128, 2, 2, 192], bf16, tag="E_sb")
                nc.scalar.activation(
                    E_sb,
                    E_ps.rearrange("p w (s c) -> p w s c",
                                   s=2)[:, :, :, 0:192],
                    AF.Exp, scale=SCALE)

                # ---- outT[c, n] (+D at col 64) = E^T @ qT65 ----
                oT_ps = ps_ot.tile([128, 2, 256], f32, tag="oT")
                for w in range(2):
                    E_hi = E_sb[:, w, 0, :]      # [128 d-hi, 192]
                    E_lo = E_sb[0:64, w, 1, :]   # [64 d-lo, 192]
                    q65h = qT_sb[:, t, w, 0:65]
                    q65l = qTlo_sb[:, t, w, 0:65]
                    nc.tensor.matmul(oT_ps[:, w, 0:65], E_hi[:, 0:128],
                                     q65h, start=True, stop=False)
                    nc.tensor.matmul(oT_ps[:, w, 0:65], E_lo[:, 0:128],
                                     q65l, start=False, stop=True)
                    nc.tensor.matmul(oT_ps[0:64, w, 128:193],
                                     E_hi[:, 128:192], q65h, start=True,
                                     stop=False)
                    nc.tensor.matmul(oT_ps[0:64, w, 128:193],
                                     E_lo[:, 128:192], q65l, start=False,
                                     stop=True)

                # ---- 1/D, then normalize: rD broadcast along tokens
                # (0-stride inner dim); one tensor_mul per hi/lo block ----
                rD_sb = sbs.tile([128, 2, 2], f32, tag="rD_sb", name="rD_sb")
                nc.vector.reciprocal_approx_fast(
                    out=rD_sb[:, :, 0:1], in_=oT_ps[:, :, 64:65])
                nc.vector.reciprocal_approx_fast(
                    out=rD_sb[0:64, :, 1:2], in_=oT_ps[0:64, :, 192:193])
                rd_h = rD_sb[:, :, 0:1]
                rd_h = bass.AP(tensor=rd_h.tensor, offset=rd_h.offset,
                               ap=[rd_h.ap[0], rd_h.ap[1], [0, WIN]])
                rd_l = rD_sb[0:64, :, 1:2]
                rd_l = bass.AP(tensor=rd_l.tensor, offset=rd_l.offset,
                               ap=[rd_l.ap[0], rd_l.ap[1], [0, WIN]])
                nc.vector.tensor_mul(outT_sb[:, t, :, :],
                                     oT_ps[:, :, 0:64], rd_h)
                nc.vector.tensor_mul(outTlo_sb[:, t, :, :],
                                     oT_ps[0:64, :, 128:192], rd_l)

                # ---- proj + bias ----
                y_ps = ps_one.tile([128, 256], f32, tag="y")
                nc.tensor.matmul(y_ps[:, 0:192],
                                 outT_sb[:, t, :, :], wp_hi,
                                 start=True, stop=False)
                nc.tensor.matmul(y_ps[:, 0:192],
                                 outTlo_sb[:, t, :, :], wp_lo,
                                 start=False, stop=True)
                nc.vector.tensor_add(y_sb[:, t, :], y_ps[:, 0:192], b_sb)

            def close_chunk(ci):
                st = chunk_state.pop(ci)
                row0 = ci * ch * TOK_TILE
                y_chunk = y_d[row0:row0 + ch * TOK_TILE, :].rearrange(
                    "(t p) c -> p t c", p=TOK_TILE)
                nc.gpsimd.dma_start(out=y_chunk, in_=st["y_sb"])

            def get_chunk(ci):
                if ci not in chunk_state:
                    chunk_state[ci] = open_chunk(ci)
                return chunk_state[ci]

            # software pipeline: front half runs one pair ahead
            front_half(get_chunk(0), 0)
            for p in range(n_pairs):
                ci, tp = divmod(p, n_pairs_per_chunk)
                st = get_chunk(ci)
                back_half(st, 2 * tp)
                if p + 1 < n_pairs:
                    ci2, tp2 = divmod(p + 1, n_pairs_per_chunk)
                    front_half(get_chunk(ci2), 2 * tp2)
                back_half(st, 2 * tp + 1)
                if tp == n_pairs_per_chunk - 1:
                    close_chunk(ci)

    nc.compile()
    return nc


def _get_nc(length=L, n_cores=N_CORES):
    key = (length, n_cores)
    if key not in _CACHE:
        _CACHE[key] = _build(length, n_cores)
    return _CACHE[key]


def kernel(x, w_qkv, w_proj, b_proj, H=None, W=None, **_unused):
    from concourse.bass_utils import run_bass_kernel_spmd

    x = np.asarray(x, dtype=np.float32)
    w_qkv = np.asarray(w_qkv, dtype=np.float32)
    w_proj = np.asarray(w_proj, dtype=np.float32)
    b_proj = np.asarray(b_proj, dtype=np.float32)
    B, length, c = x.shape
    assert B == N_CORES and c == C

    nc = _get_nc(length, N_CORES)
    in_maps = [
        {"x": np.ascontiguousarray(x[b]), "w_qkv": w_qkv, "w_proj": w_proj,
         "b_proj": b_proj}
        for b in range(B)
    ]
    res = run_bass_kernel_spmd(nc, in_maps, list(range(N_CORES)))
    return np.stack([res.results[b]["y"] for b in range(B)], axis=0)


def _np_ref(x, w_qkv, w_proj, b_proj):
    qkv = x @ w_qkv
    B_, L_, _ = x.shape
    qkv = qkv.reshape(B_, L_ // 64, 64, 3, C)
    q, k, v = qkv[..., 0, :], qkv[..., 1, :] * SCALE, qkv[..., 2, :]
    attn = np.einsum('bwnc,bwnd->bwcd', k, v)
    attn = np.exp(attn)
    attn = attn / attn.sum(-1, keepdims=True)
    out = np.einsum('bwcd,bwnd->bwnc', attn, q).reshape(B_, L_, C)
    return out @ w_proj + b_proj


if __name__ == "__main__":
    length = int(os.environ.get("K_LEN", 2 * CH * TOK_TILE))
    rng = np.random.default_rng(0)
    x = rng.standard_normal((N_CORES, length, C), dtype=np.float32)
    w_qkv = (rng.standard_normal((C, 3 * C)) * 0.02).astype(np.float32)
    w_proj = (rng.standard_normal((C, C)) * 0.02).astype(np.float32)
    b_proj = (rng.standard_normal((C,)) * 0.02).astype(np.float32)

    expected = _np_ref(x, w_qkv, w_proj, b_proj)
    got = kernel(x, w_qkv, w_proj, b_proj)
    err = np.abs(got - expected).max()
    rel = np.linalg.norm(got - expected) / np.linalg.norm(expected)
    print(f"mini test: max abs err {err:.3e}  rel_fro {rel:.3e}")



# revision 8
# speedup vs baseline: 30.5512x; 1.0127x over previous
"""Trainium2 Bass kernel v2 for windowed channel-attention (nn_ChannelAttention2).

Reference computation (per batch element b, one NeuronCore each):
    qkv = x @ w_qkv                    # [L, 3C], L = 36864, C = 192
    per 64-token window w:
        q, k, v = qkv[w]               # [64, C] each
        E = exp(scale * k^T v)         # [C, C]  (attnT[d, c], softmax over d)
        D[c] = sum_d E[d, c]
        out[w][n, c] = (sum_d q[n, d] E[d, c]) / D[c]
    y = out @ w_proj + b_proj

v2 changes vs baseline (all-fp32; true device time ~4 ms -> ~1.2 ms):
  - all matmul operands bf16 (fp32 is 4 cycles/row on the PE, bf16 is 1;
    rel-err budget 2e-2 >> bf16 error; measured rel_fro ~1e-3)
  - attention output computed directly transposed: outT[c, n] = E^T qT65 via
    lhsT=E (stationary), rhs=qT65 (moving) -> no separate out transposes
  - softmax denominator via ones-column appended to qT: outT[c, 64] = D[c]
    lands with c on partitions -> tiny reciprocal, then normalization is one
    tensor_mul per c-block with 1/D broadcast along tokens (0-stride AP),
    fused into the PSUM->SBUF evacuation
  - batched evacuations (one op per tile pair, strided APs) to amortize the
    per-op engine init bubble (ACT 172-222 cyc, DVE 58-120 cyc)
  - engine balance: exp + xT/kv evacs on ACT; qT evacs, recip, muls, y on DVE;
    memsets + chunk HBM DMAs on GPSIMD (SWDGE) keeping the SP FIFO free for
    the latency-critical k/v repacks
  - software-pipelined emission (front half of pair p+1 issued mid-pair p)
    to avoid head-of-line blocking in the strict-FIFO engine queues; PSUM
    banks: xq/qT rotation 2, kv 1, E 2, outT 2 (double-buffered), y 1

Matmul operands must live at partition base 0 (tile_position wedges this
stack), so k/v are repacked window-major via SBUF->SBUF DMA as in baseline.
tensor_tensor(divide) crashes the walrus birverifier -> reciprocal + mul.
"""

import os

import numpy as np

C = 192
L = 36864
N_CORES = 8
WIN = 64
TOK_TILE = 128  # 2 windows
CH = 8          # token tiles per chunk (DMA batching granularity)
SCALE = float((C // 8) ** -0.5)

_CACHE = {}


def _build(length=L, n_cores=N_CORES):
    import concourse.bass as bass
    import concourse.mybir as mybir
    import concourse.tile as tile
    from concourse import bacc
    from concourse.masks import make_identity

    f32 = mybir.dt.float32
    bf16 = mybir.dt.bfloat16
    AF = mybir.ActivationFunctionType

    n_tiles = length // TOK_TILE
    ch = min(CH, n_tiles)
    n_chunks = n_tiles // ch
    assert n_chunks * ch == n_tiles

    nc = bacc.Bacc("TRN2", target_bir_lowering=False, debug=False,
                   num_devices=n_cores)
    x_d = nc.declare_dram_parameter("x", [length, C], f32, isOutput=False)
    wqkv_d = nc.declare_dram_parameter("w_qkv", [C, 3 * C], f32, isOutput=False)
    wp_d = nc.declare_dram_parameter("w_proj", [C, C], f32, isOutput=False)
    bp_d = nc.declare_dram_parameter("b_proj", [C], f32, isOutput=False)
    y_d = nc.declare_dram_parameter("y", [length, C], f32, isOutput=True)

    with tile.TileContext(nc) as tc:
        with (
            tc.tile_pool(name="singles", bufs=1) as singles,
            tc.tile_pool(name="sb", bufs=2) as sb,
            tc.tile_pool(name="sbs", bufs=4) as sbs,
            tc.tile_pool(name="ps_a", bufs=2, space="PSUM") as ps_a,
            tc.tile_pool(name="ps_ot", bufs=2, space="PSUM") as ps_ot,
            tc.tile_pool(name="ps_kv", bufs=1, space="PSUM") as ps_kv,
            tc.tile_pool(name="ps_one", bufs=1, space="PSUM") as ps_one,
        ):
            # ---- constants / weights (loaded once, converted to bf16) ----
            ident = singles.tile([128, 128], f32)
            make_identity(nc, ident)

            wqkv_f32h = singles.tile([128, 3 * C], f32)
            nc.sync.dma_start(out=wqkv_f32h, in_=wqkv_d[0:128, :])
            wqkv_f32l = singles.tile([64, 3 * C], f32)
            nc.sync.dma_start(out=wqkv_f32l, in_=wqkv_d[128:192, :])
            wp_f32h = singles.tile([128, C], f32)
            nc.sync.dma_start(out=wp_f32h, in_=wp_d[0:128, :])
            wp_f32l = singles.tile([64, C], f32)
            nc.sync.dma_start(out=wp_f32l, in_=wp_d[128:192, :])

            wqkv_hi = singles.tile([128, 3 * C], bf16)
            nc.vector.tensor_copy(wqkv_hi, wqkv_f32h)
            wqkv_lo = singles.tile([64, 3 * C], bf16)
            nc.vector.tensor_copy(wqkv_lo, wqkv_f32l)
            wp_hi = singles.tile([128, C], bf16)
            nc.vector.tensor_copy(wp_hi, wp_f32h)
            wp_lo = singles.tile([64, C], bf16)
            nc.vector.tensor_copy(wp_lo, wp_f32l)

            b_sb = singles.tile([128, C], f32)
            nc.gpsimd.dma_start(
                out=b_sb,
                in_=bass.AP(tensor=bp_d, offset=0, ap=[[0, 128], [1, C]]))

            n_pairs_per_chunk = ch // 2
            n_pairs = n_tiles // 2
            chunk_state = {}

            def open_chunk(ci):
                row0 = ci * ch * TOK_TILE
                x_chunk = x_d[row0:row0 + ch * TOK_TILE, :].rearrange(
                    "(t p) c -> p t c", p=TOK_TILE)
                st = {}
                st["x_sb"] = sb.tile([128, ch, 256], f32, tag="x_sb")
                nc.gpsimd.memset(st["x_sb"][:, :, 192:256], 0.0)
                nc.gpsimd.dma_start(out=st["x_sb"][:, :, 0:192], in_=x_chunk)
                st["xT_sb"] = sb.tile([128, ch, 256], bf16, tag="xT_sb")
                st["kv_sb"] = sb.tile([128, ch, 2 * C], bf16, tag="kv_sb")
                st["kv2_sb"] = sb.tile([64, ch, 2, 448], bf16, tag="kv2_sb")
                nc.gpsimd.memset(st["kv2_sb"][:, :, :, 384:448], 0.0)
                st["qT_sb"] = sb.tile([128, ch, 2, 66], bf16, tag="qT_sb")
                st["qTlo_sb"] = sb.tile([64, ch, 2, 66], bf16, tag="qTlo_sb")
                st["outT_sb"] = sb.tile([128, ch, 2, WIN], bf16, tag="outT_sb")
                st["outTlo_sb"] = sb.tile([64, ch, 2, WIN], bf16,
                                          tag="outTlo_sb")
                st["y_sb"] = sb.tile([128, ch, C], f32, tag="y_sb")
                nc.gpsimd.memset(st["qT_sb"][:, :, :, 64:65], 1.0)
                nc.gpsimd.memset(st["qTlo_sb"][:, :, :, 64:65], 1.0)
                return st

            def front_half(st, t0):
                x_sb = st["x_sb"]
                xT_sb = st["xT_sb"]
                kv_sb = st["kv_sb"]
                qT_sb = st["qT_sb"]
                qTlo_sb = st["qTlo_sb"]
                tsl = slice(t0, t0 + 2)
                # ---- x tiles -> xT (PE transpose, fp32), evac converts ----
                xq_ps = ps_a.tile([128, 2, 256], f32, tag="xqT")
                for i in range(2):
                    nc.tensor.transpose(xq_ps[:, i, 0:128],
                                        x_sb[:, t0 + i, 0:128], ident)
                    nc.tensor.transpose(xq_ps[:, i, 128:256],
                                        x_sb[:, t0 + i, 128:256], ident)
                nc.scalar.copy(xT_sb[:, tsl, :], xq_ps)

                # ---- k, v token-major: out[tok, 2C]; per-tile evac +
                # repack so the attn pipeline starts sooner ----
                kv2_sb = st["kv2_sb"]
                for i in range(2):
                    kv_ps = ps_kv.tile([128, 512], f32, tag="kv")
                    xT_hi = xT_sb[:, t0 + i, 0:128]
                    xT_lo = xT_sb[0:64, t0 + i, 128:256]
                    nc.tensor.matmul(kv_ps[:, 0:384], xT_hi,
                                     wqkv_hi[:, 192:576], start=True,
                                     stop=False)
                    nc.tensor.matmul(kv_ps[:, 0:384], xT_lo,
                                     wqkv_lo[:, 192:576], start=False,
                                     stop=True)
                    nc.scalar.copy(kv_sb[:, t0 + i, :], kv_ps[:, 0:384])
                    nc.sync.dma_start(
                        out=kv2_sb[:, t0 + i, 0, 0:384],
                        in_=kv_sb[0:64, t0 + i, :])
                    nc.sync.dma_start(
                        out=kv2_sb[:, t0 + i, 1, 0:384],
                        in_=kv_sb[64:128, t0 + i, :])

                # ---- qT channel-major: [e, tok]; e-lo at rows 0:64 ----
                qT_ps = ps_a.tile([128, 2, 256], f32, tag="xqT")
                for i in range(2):
                    xT_hi = xT_sb[:, t0 + i, 0:128]
                    xT_lo = xT_sb[0:64, t0 + i, 128:256]
                    nc.tensor.matmul(qT_ps[:, i, 0:128],
                                     wqkv_hi[:, 0:128], xT_hi,
                                     start=True, stop=False)
                    nc.tensor.matmul(qT_ps[:, i, 0:128],
                                     wqkv_lo[:, 0:128], xT_lo,
                                     start=False, stop=True)
                    nc.tensor.matmul(qT_ps[0:64, i, 128:256],
                                     wqkv_hi[:, 128:192], xT_hi,
                                     start=True, stop=False)
                    nc.tensor.matmul(qT_ps[0:64, i, 128:256],
                                     wqkv_lo[:, 128:192], xT_lo,
                                     start=False, stop=True)
                nc.vector.tensor_copy(
                    qT_sb[:, tsl, :, 0:64],
                    qT_ps[:, :, 0:128].rearrange(
                        "p t (w n) -> p t w n", w=2))
                nc.vector.tensor_copy(
                    qTlo_sb[:, tsl, :, 0:64],
                    qT_ps[0:64, :, 128:256].rearrange(
                        "p t (w n) -> p t w n", w=2))


            def back_half(st, t):
                kv2_sb = st["kv2_sb"]
                qT_sb = st["qT_sb"]
                qTlo_sb = st["qTlo_sb"]
                outT_sb = st["outT_sb"]
                outTlo_sb = st["outTlo_sb"]
                y_sb = st["y_sb"]
                # ---- attnT per window: E_ps[d, c] = sum_n v[n,d] k[n,c] ----
                E_ps = ps_one.tile([128, 2, 512], f32, tag="E")
                for w in range(2):
                    k_sl = kv2_sb[:, t, w, 0:192]
                    v_hi = kv2_sb[:, t, w, 192:320]
                    v_lo = kv2_sb[:, t, w, 320:448]
                    nc.tensor.matmul(E_ps[:, w, 0:192], v_hi, k_sl,
                                     start=True, stop=True)
                    nc.tensor.matmul(E_ps[:, w, 256:448], v_lo, k_sl,
                                     start=True, stop=True)

                # ---- exp (scale folded in); one op covers hi+lo ----
                E_sb = sbs.tile([128, 2, 2, 192], bf16, tag="E_sb")
                nc.scalar.activation(
                    E_sb,
                    E_ps.rearrange("p w (s c) -> p w s c",
                                   s=2)[:, :, :, 0:192],
                    AF.Exp, scale=SCALE)

                # ---- outT[c, n] (+D at col 64) = E^T @ qT65 ----
                oT_ps = ps_ot.tile([128, 2, 256], f32, tag="oT")
                for w in range(2):
                    E_hi = E_sb[:, w, 0, :]      # [128 d-hi, 192]
                    E_lo = E_sb[0:64, w, 1, :]   # [64 d-lo, 192]
                    q65h = qT_sb[:, t, w, 0:65]
                    q65l = qTlo_sb[:, t, w, 0:65]
                    nc.tensor.matmul(oT_ps[:, w, 0:65], E_hi[:, 0:128],
                                     q65h, start=True, stop=False)
                    nc.tensor.matmul(oT_ps[:, w, 0:65], E_lo[:, 0:128],
                                     q65l, start=False, stop=True)
                    nc.tensor.matmul(oT_ps[0:64, w, 128:193],
                                     E_hi[:, 128:192], q65h, start=True,
                                     stop=False)
                    nc.tensor.matmul(oT_ps[0:64, w, 128:193],
                                     E_lo[:, 128:192], q65l, start=False,
                                     stop=True)

                # ---- 1/D, then normalize: rD broadcast along tokens
                # (0-stride inner dim); one tensor_mul per hi/lo block ----
                rD_sb = sbs.tile([128, 2, 2], f32, tag="rD_sb", name="rD_sb")
                nc.vector.reciprocal_approx_fast(
                    out=rD_sb[:, :, 0:1], in_=oT_ps[:, :, 64:65])
                nc.vector.reciprocal_approx_fast(
                    out=rD_sb[0:64, :, 1:2], in_=oT_ps[0:64, :, 192:193])
                rd_h = rD_sb[:, :, 0:1]
                rd_h = bass.AP(tensor=rd_h.tensor, offset=rd_h.offset,
                               ap=[rd_h.ap[0], rd_h.ap[1], [0, WIN]])
                rd_l = rD_sb[0:64, :, 1:2]
                rd_l = bass.AP(tensor=rd_l.tensor, offset=rd_l.offset,
                               ap=[rd_l.ap[0], rd_l.ap[1], [0, WIN]])
                nc.vector.tensor_mul(outT_sb[:, t, :, :],
                                     oT_ps[:, :, 0:64], rd_h)
                nc.vector.tensor_mul(outTlo_sb[:, t, :, :],
                                     oT_ps[0:64, :, 128:192], rd_l)

                # ---- proj + bias ----
                y_ps = ps_one.tile([128, 256], f32, tag="y")
                nc.tensor.matmul(y_ps[:, 0:192],
                                 outT_sb[:, t, :, :], wp_hi,
                                 start=True, stop=False)
                nc.tensor.matmul(y_ps[:, 0:192],
                                 outTlo_sb[:, t, :, :], wp_lo,
                                 start=False, stop=True)
                nc.vector.tensor_add(y_sb[:, t, :], y_ps[:, 0:192], b_sb)

            def close_chunk(ci):
                st = chunk_state.pop(ci)
                row0 = ci * ch * TOK_TILE
                y_chunk = y_d[row0:row0 + ch * TOK_TILE, :].rearrange(
                    "(t p) c -> p t c", p=TOK_TILE)
                nc.gpsimd.dma_start(out=y_chunk, in_=st["y_sb"])

            def get_chunk(ci):
                if ci not in chunk_state:
                    chunk_state[ci] = open_chunk(ci)
                return chunk_state[ci]

            # software pipeline: front half runs one pair ahead
            front_half(get_chunk(0), 0)
            for p in range(n_pairs):
                ci, tp = divmod(p, n_pairs_per_chunk)
                st = get_chunk(ci)
                back_half(st, 2 * tp)
                if p + 1 < n_pairs:
                    ci2, tp2 = divmod(p + 1, n_pairs_per_chunk)
                    front_half(get_chunk(ci2), 2 * tp2)
                back_half(st, 2 * tp + 1)
                if tp == n_pairs_per_chunk - 1:
                    close_chunk(ci)

    nc.compile()
    return nc


def _get_nc(length=L, n_cores=N_CORES):
    key = (length, n_cores)
    if key not in _CACHE:
        _CACHE[key] = _build(length, n_cores)
    return _CACHE[key]


def kernel(x, w_qkv, w_proj, b_proj, H=None, W=None, **_unused):
    from concourse.bass_utils import run_bass_kernel_spmd

    x = np.asarray(x, dtype=np.float32)
    w_qkv = np.asarray(w_qkv, dtype=np.float32)
    w_proj = np.asarray(w_proj, dtype=np.float32)
    b_proj = np.asarray(b_proj, dtype=np.float32)
    B, length, c = x.shape
    assert B == N_CORES and c == C

    nc = _get_nc(length, N_CORES)
    in_maps = [
        {"x": np.ascontiguousarray(x[b]), "w_qkv": w_qkv, "w_proj": w_proj,
         "b_proj": b_proj}
        for b in range(B)
    ]
    res = run_bass_kernel_spmd(nc, in_maps, list(range(N_CORES)))
    return np.stack([res.results[b]["y"] for b in range(B)], axis=0)


def _np_ref(x, w_qkv, w_proj, b_proj):
    qkv = x @ w_qkv
    B_, L_, _ = x.shape
    qkv = qkv.reshape(B_, L_ // 64, 64, 3, C)
    q, k, v = qkv[..., 0, :], qkv[..., 1, :] * SCALE, qkv[..., 2, :]
    attn = np.einsum('bwnc,bwnd->bwcd', k, v)
    attn = np.exp(attn)
    attn = attn / attn.sum(-1, keepdims=True)
    out = np.einsum('bwcd,bwnd->bwnc', attn, q).reshape(B_, L_, C)
    return out @ w_proj + b_proj


if __name__ == "__main__":
    length = int(os.environ.get("K_LEN", 2 * CH * TOK_TILE))
    rng = np.random.default_rng(0)
    x = rng.standard_normal((N_CORES, length, C), dtype=np.float32)
    w_qkv = (rng.standard_normal((C, 3 * C)) * 0.02).astype(np.float32)
    w_proj = (rng.standard_normal((C, C)) * 0.02).astype(np.float32)
    b_proj = (rng.standard_normal((C,)) * 0.02).astype(np.float32)

    expected = _np_ref(x, w_qkv, w_proj, b_proj)
    got = kernel(x, w_qkv, w_proj, b_proj)
    err = np.abs(got - expected).max()
    rel = np.linalg.norm(got - expected) / np.linalg.norm(expected)
    print(f"mini test: max abs err {err:.3e}  rel_fro {rel:.3e}")
